# revision 1
# baseline (speedup 1.0000x reference)
"""Contrastive loss kernel for Trainium2, 8 NeuronCores, data-parallel over node rows.

Strategy (per core c, shard rows R_c = c*1024 .. c*1024+1024), gather-free:
  - Host pre-casts x to bf16 and uploads a per-core log-mask
    lnm[p, s, c] = 0 if c is a negative of row (c*1024 + s*128 + p) else -20,
    stored as fp8_e4m3 (exact for both values).  Since top-k indices are
    distinct within a row, the mask is binary.
  - On-chip: normalize full x (ACT square + DVE reduce + recip/sqrt), scale to
    z in bf16 (split ACT/DVE), DMA-transpose into z1T [256, 8192] for the PE.
  - Slab loop (8 slabs x 128 rows): Gram chunk [128, 2048] = z1s @ z1^T on PE
    (bf16, K=256).  Then the selection is FUSED arithmetic, no gather:
        DVE: Y = G_psum + lnm_chunk          (PSUM eviction + mask in one pass)
        ACT: ex = exp(2*Y), accum_out += row-sum   (exp + reduce in one pass)
    Unselected columns contribute exp(2*(sim-20)) ~ e^-38 ~ 0.
  - Positives: per-tile fused dot via DVE tensor_tensor_reduce on bf16
    shard tiles; pos = exp(2 * dot * rx * ry); ln(pos) taken analytically.
  - loss = ln(pos + neg + eps) - 2*pos_arg; host averages the 8 cores' rows.
"""
import sys

sys.path.insert(0, "/opt/trn_rl_repo")

from contextlib import ExitStack

import numpy as np
import ml_dtypes

import concourse.bacc as bacc
import concourse.mybir as mybir
import concourse.tile as tile
from concourse.bass_utils import run_bass_kernel_spmd

N_NODES = 8192
D = 256
K_NEG = 64
N_CORES = 8
RPC = N_NODES // N_CORES      # rows per core = 1024
SLABS = RPC // 128            # 8 slabs of 128 rows
NT = N_NODES // 128           # 64 x-tiles of [128, 256]
NTS = RPC // 128              # 8 shard tiles
G = 8                         # tiles per x-group
CHUNK = 2048                  # Gram eviction chunk (4 PSUM banks)
NCH = N_NODES // CHUNK        # 4 chunks per slab
TAU_INV = float(1.0 / (0.5 + 1e-10))
EPS = 1e-5
MASK_OFF = -20.0              # ln-mask "minus infinity"
GATHER_SLABS = (2, 4, 6)      # slabs whose selection runs on GPSIMD ap_gather
NGS = len(GATHER_SLABS)

F32 = mybir.dt.float32
BF16 = mybir.dt.bfloat16
MASK_DT = mybir.dt.float8e4
MASK_NP = ml_dtypes.float8_e4m3

_PROG = None


def _build_program():
    nc = bacc.Bacc("TRN2", target_bir_lowering=False, debug=False,
                   num_devices=N_CORES)

    xb_d = nc.dram_tensor("xb", [N_NODES, D], BF16, kind="ExternalInput")
    xs_d = nc.dram_tensor("xs", [RPC, D], BF16, kind="ExternalInput")
    ys_d = nc.dram_tensor("ys", [RPC, D], BF16, kind="ExternalInput")
    lnm_d = nc.dram_tensor("lnm", [128, (SLABS - NGS) * N_NODES], MASK_DT,
                           kind="ExternalInput")
    idx_d = nc.dram_tensor("idx", [NGS, 128, K_NEG], mybir.dt.int16,
                           kind="ExternalInput")
    loss_d = nc.dram_tensor("loss", [128, SLABS], F32, kind="ExternalOutput")

    AF = mybir.ActivationFunctionType
    ALU = mybir.AluOpType

    with tile.TileContext(nc) as tc, ExitStack() as ctx:
        big = ctx.enter_context(tc.tile_pool(name="big", bufs=1))
        xg_pool = ctx.enter_context(tc.tile_pool(name="xg", bufs=2))
        sqpool = ctx.enter_context(tc.tile_pool(name="sqpool", bufs=2))
        zrow_pool = ctx.enter_context(tc.tile_pool(name="zrow", bufs=2))
        mpool = ctx.enter_context(tc.tile_pool(name="mpool", bufs=2))
        ypool = ctx.enter_context(tc.tile_pool(name="ypool", bufs=2))
        expool = ctx.enter_context(tc.tile_pool(name="expool", bufs=2))
        epool = ctx.enter_context(tc.tile_pool(name="epool", bufs=2))
        psum = ctx.enter_context(tc.tile_pool(name="psum", bufs=2, space="PSUM"))

        # ---------------- shard loads ----------------
        xsb = big.tile([128, NTS, D], BF16)
        nc.sync.dma_start(out=xsb, in_=xs_d.ap().rearrange("(t p) d -> p t d", p=128))
        ysb = big.tile([128, NTS, D], BF16)
        nc.sync.dma_start(out=ysb, in_=ys_d.ap().rearrange("(t p) d -> p t d", p=128))

        # ---------------- shard norms + positive dots ----------------
        SSx = big.tile([128, NTS], F32)
        sqs = sqpool.tile([128, G, D], BF16, tag="sq")
        nc.vector.tensor_mul(sqs[:, 0:NTS, :].rearrange("p a b -> p (a b)"),
                             xsb.rearrange("p a b -> p (a b)"),
                             xsb.rearrange("p a b -> p (a b)"))
        nc.vector.tensor_reduce(out=SSx, in_=sqs[:, 0:NTS, :],
                                axis=mybir.AxisListType.X, op=ALU.add)
        SSy = big.tile([128, NTS], F32)
        sqy = sqpool.tile([128, G, D], BF16, tag="sq")
        nc.vector.tensor_mul(sqy[:, 0:NTS, :].rearrange("p a b -> p (a b)"),
                             ysb.rearrange("p a b -> p (a b)"),
                             ysb.rearrange("p a b -> p (a b)"))
        nc.vector.tensor_reduce(out=SSy, in_=sqy[:, 0:NTS, :],
                                axis=mybir.AxisListType.X, op=ALU.add)

        SSxi = big.tile([128, NTS], F32)
        nc.vector.reciprocal(SSxi, SSx)
        Rx = big.tile([128, NTS], F32)
        nc.scalar.activation(Rx, SSxi, AF.Sqrt)
        SSyi = big.tile([128, NTS], F32)
        nc.vector.reciprocal(SSyi, SSy)
        Ry = big.tile([128, NTS], F32)
        nc.scalar.activation(Ry, SSyi, AF.Sqrt)

        # positive dots: bf16 elementwise product + f32 reduce
        xyp = sqpool.tile([128, G, D], BF16, tag="sq")
        nc.vector.tensor_mul(xyp[:, 0:NTS, :].rearrange("p a b -> p (a b)"),
                             xsb.rearrange("p a b -> p (a b)"),
                             ysb.rearrange("p a b -> p (a b)"))
        DXY = big.tile([128, NTS], F32)
        nc.vector.tensor_reduce(out=DXY, in_=xyp[:, 0:NTS, :],
                                axis=mybir.AxisListType.X, op=ALU.add)
        PA = big.tile([128, NTS], F32)
        nc.vector.tensor_mul(PA, DXY, Rx)
        PA2 = big.tile([128, NTS], F32)
        nc.vector.tensor_mul(PA2, PA, Ry)
        POS = big.tile([128, NTS], F32)
        nc.scalar.activation(POS, PA2, AF.Exp, scale=TAU_INV)

        # ---------------- z1sT build (shard lhsT) ----------------
        z1sT0 = big.tile([128, RPC], BF16)
        z1sT1 = big.tile([128, RPC], BF16)
        zsA = zrow_pool.tile([128, G, 128], BF16, tag="zrA")
        zsB = zrow_pool.tile([128, G, 128], BF16, tag="zrB")
        for t in range(NTS):
            nc.scalar.activation(zsA[:, t, :], xsb[:, t, 0:128], AF.Copy,
                                 scale=Rx[:, t:t + 1])
            nc.vector.tensor_scalar(out=zsB[:, t, :], in0=xsb[:, t, 128:256],
                                    scalar1=Rx[:, t:t + 1], scalar2=None,
                                    op0=ALU.mult)
        nc.sync.dma_start(out=z1sT0.rearrange("p (b q) -> p b q", q=128),
                          in_=zsA.rearrange("p a b -> p (a b)"), transpose=True)
        nc.sync.dma_start(out=z1sT1.rearrange("p (b q) -> p b q", q=128),
                          in_=zsB.rearrange("p a b -> p (a b)"), transpose=True)

        # ---------------- full-x normalize + transpose, streamed in groups ----
        x_r = xb_d.ap().rearrange("(t p) d -> p t d", p=128)
        z1T0 = big.tile([128, N_NODES], BF16)  # d in [0,128)
        z1T1 = big.tile([128, N_NODES], BF16)  # d in [128,256)
        SS = big.tile([128, NT], F32)
        SSi = big.tile([128, NT], F32)
        R = big.tile([128, NT], F32)
        for g in range(NT // G):
            sl = slice(g * G, (g + 1) * G)
            xg = xg_pool.tile([128, G, D], BF16, tag="xg")
            nc.sync.dma_start(out=xg, in_=x_r[:, sl, :])
            sq = sqpool.tile([128, G, D], BF16, tag="sq")
            nc.vector.tensor_mul(sq.rearrange("p a b -> p (a b)"),
                                 xg.rearrange("p a b -> p (a b)"),
                                 xg.rearrange("p a b -> p (a b)"))
            nc.vector.tensor_reduce(out=SS[:, sl], in_=sq,
                                    axis=mybir.AxisListType.X, op=ALU.add)
            nc.vector.reciprocal(SSi[:, sl], SS[:, sl])
            nc.scalar.activation(R[:, sl], SSi[:, sl], AF.Sqrt)
            zrA = zrow_pool.tile([128, G, 128], BF16, tag="zrA")
            zrB = zrow_pool.tile([128, G, 128], BF16, tag="zrB")
            for tl in range(G):
                t = g * G + tl
                # split the scaling across ACT (first half) and DVE (second)
                nc.scalar.activation(zrA[:, tl, :], xg[:, tl, 0:128], AF.Copy,
                                     scale=R[:, t:t + 1])
                nc.vector.tensor_scalar(out=zrB[:, tl, :], in0=xg[:, tl, 128:256],
                                        scalar1=R[:, t:t + 1], scalar2=None,
                                        op0=ALU.mult)
            nc.sync.dma_start(
                out=z1T0[:, g * G * 128:(g + 1) * G * 128].rearrange(
                    "p (b q) -> p b q", q=128),
                in_=zrA.rearrange("p a b -> p (a b)"), transpose=True)
            nc.sync.dma_start(
                out=z1T1[:, g * G * 128:(g + 1) * G * 128].rearrange(
                    "p (b q) -> p b q", q=128),
                in_=zrB.rearrange("p a b -> p (a b)"), transpose=True)

        # ---------------- slab loop: Gram + fused mask/exp/reduce ----------
        idx_sb = big.tile([128, NGS, K_NEG], mybir.dt.int16)
        nc.sync.dma_start(out=idx_sb, in_=idx_d.ap().rearrange("s p k -> p s k"))
        Ubig = big.tile([128, NGS, RPC], F32)   # ap_gather output per gather-slab
        NEGC = big.tile([128, SLABS * NCH], F32)  # per-chunk partial sums
        nc.vector.memset(NEGC, 0.0)
        mask_slot = 0
        for s in range(SLABS):
            is_gather = s in GATHER_SLABS
            gi = GATHER_SLABS.index(s) if is_gather else -1
            if is_gather:
                E = epool.tile([128, N_NODES], F32, tag="E")
            else:
                msk = mpool.tile([128, N_NODES], MASK_DT, tag="m")
                nc.sync.dma_start(
                    out=msk,
                    in_=lnm_d.ap()[:, mask_slot * N_NODES:(mask_slot + 1) * N_NODES])
                mask_slot += 1
            lhs0 = z1sT0[:, s * 128:(s + 1) * 128]
            lhs1 = z1sT1[:, s * 128:(s + 1) * 128]
            for c4 in range(NCH):
                ps = psum.tile([128, CHUNK], F32, tag="ps")
                for j in range(CHUNK // 512):
                    col = c4 * CHUNK + j * 512
                    nc.tensor.matmul(ps[:, j * 512:(j + 1) * 512],
                                     lhsT=lhs0, rhs=z1T0[:, col:col + 512],
                                     start=True, stop=False)
                    nc.tensor.matmul(ps[:, j * 512:(j + 1) * 512],
                                     lhsT=lhs1, rhs=z1T1[:, col:col + 512],
                                     start=False, stop=True)
                if is_gather:
                    nc.scalar.copy(E[:, c4 * CHUNK:(c4 + 1) * CHUNK], ps)
                else:
                    Y = ypool.tile([128, CHUNK], BF16, tag="y")
                    nc.vector.tensor_tensor(
                        out=Y, in0=ps, in1=msk[:, c4 * CHUNK:(c4 + 1) * CHUNK],
                        op=ALU.add)
                    ex = expool.tile([128, CHUNK], BF16, tag="ex")
                    nc.scalar.activation(
                        ex, Y, AF.Exp, scale=TAU_INV,
                        accum_out=NEGC[:, s * NCH + c4:s * NCH + c4 + 1])
            if is_gather:
                nc.gpsimd.ap_gather(out_ap=Ubig[:, gi, :], in_ap=E,
                                    idxs_ap=idx_sb[:, gi, :],
                                    channels=128, num_elems=N_NODES, d=1,
                                    num_idxs=RPC)

        # gather-slab finish: extract own blocks, exp, reduce into NEGC col 0
        EX = big.tile([128, NGS, K_NEG], F32)
        for q in range(16):
            nc.sync.dma_start(
                out=EX[q:128:16, :, :],
                in_=Ubig[q:128:16, :, q * K_NEG:(q + 1) * K_NEG])
        EEX = big.tile([128, NGS, K_NEG], F32)
        nc.scalar.activation(EEX.rearrange("p a b -> p (a b)"),
                             EX.rearrange("p a b -> p (a b)"),
                             AF.Exp, scale=TAU_INV)
        for gi, s in enumerate(GATHER_SLABS):
            nc.vector.tensor_reduce(out=NEGC[:, s * NCH:s * NCH + 1],
                                    in_=EEX[:, gi, :],
                                    axis=mybir.AxisListType.X, op=ALU.add)

        # ---------------- loss assembly ----------------
        NEG = big.tile([128, SLABS], F32)
        nc.vector.tensor_reduce(out=NEG, in_=NEGC.rearrange("p (s c) -> p s c", c=NCH),
                                axis=mybir.AxisListType.X,
                                op=ALU.add)
        DEN = big.tile([128, SLABS], F32)
        nc.vector.tensor_add(DEN, NEG, POS)
        DEN2 = big.tile([128, SLABS], F32)
        nc.vector.tensor_scalar_add(DEN2, DEN, EPS)
        LD = big.tile([128, SLABS], F32)
        nc.scalar.activation(LD, DEN2, AF.Ln)
        LP = big.tile([128, SLABS], F32)
        nc.vector.tensor_scalar_mul(LP, PA2, TAU_INV)
        LOSS = big.tile([128, SLABS], F32)
        nc.vector.tensor_sub(LOSS, LD, LP)
        nc.sync.dma_start(out=loss_d.ap(), in_=LOSS)

    nc.compile()
    return nc


def _get_program():
    global _PROG
    if _PROG is None:
        _PROG = _build_program()
    return _PROG


MASK_SLABS = tuple(s for s in range(SLABS) if s not in GATHER_SLABS)


def _make_mask(idx_core: np.ndarray) -> np.ndarray:
    """[1024, 64] int -> [128, NMS*8192] fp8 ln-mask (p-major, mask slabs only)."""
    nms = len(MASK_SLABS)
    idxc = idx_core.reshape(SLABS, 128, K_NEG)[list(MASK_SLABS)]
    idxc = idxc.transpose(1, 0, 2)  # [p, ms, k]
    lnm = np.full((128, nms, N_NODES), MASK_OFF, dtype=np.float32)
    pp = np.arange(128)[:, None, None]
    ss = np.arange(nms)[None, :, None]
    lnm[pp, ss, idxc] = 0.0
    return lnm.reshape(128, nms * N_NODES).astype(MASK_NP)


def _idx_layout(idx_core: np.ndarray) -> np.ndarray:
    """ap_gather index layout for the gather slabs: [NGS, 128, 64] int16."""
    out = np.empty((NGS, 128, K_NEG), dtype=np.int16)
    for o, s in enumerate(GATHER_SLABS):
        A = idx_core[s * 128:(s + 1) * 128].astype(np.int16)  # [128, 64]
        A = A.reshape(8, 16, 4, 16)        # [g, p, t, q]
        A = A.transpose(0, 3, 1, 2)        # [g, q, p, t]
        out[o] = A.reshape(128, 64)
    return out


def make_in_maps(x, y, neg_indices):
    xb = np.ascontiguousarray(x).astype(ml_dtypes.bfloat16)
    in_maps = []
    for c in range(N_CORES):
        lo, hi = c * RPC, (c + 1) * RPC
        in_maps.append({
            "xb": xb,
            "xs": xb[lo:hi],
            "ys": np.ascontiguousarray(y[lo:hi]).astype(ml_dtypes.bfloat16),
            "lnm": _make_mask(neg_indices[lo:hi]),
            "idx": _idx_layout(neg_indices[lo:hi]),
        })
    return in_maps


def _ensure_ntff_hook():
    """Register the axon NTFF profile hook (missing from this image's antenv)."""
    import types, ctypes, contextlib
    try:
        from antenv.axon_hooks import get_axon_ntff_profile_hook  # noqa
        return
    except ImportError:
        pass
    so_path = "/opt/axon/libaxon_pjrt.so"
    import os
    if not os.path.exists(so_path):
        return
    lib = ctypes.CDLL(so_path)
    if not hasattr(lib, "axon_start_nrt_profile"):
        return
    lib.axon_start_nrt_profile.argtypes = [ctypes.POINTER(ctypes.c_int64),
                                           ctypes.c_size_t]
    lib.axon_start_nrt_profile.restype = ctypes.c_int64
    lib.axon_stop_nrt_profile.argtypes = [ctypes.c_char_p]
    lib.axon_stop_nrt_profile.restype = ctypes.c_int64

    @contextlib.contextmanager
    def _hook(output_dir, device_ids):
        import jax
        jax.devices()
        if device_ids:
            ids = (ctypes.c_int64 * len(device_ids))(*device_ids)
            rc = lib.axon_start_nrt_profile(ids, len(device_ids))
        else:
            rc = lib.axon_start_nrt_profile(None, 0)
        if rc != 0:
            raise RuntimeError(f"axon_start_nrt_profile rc={rc}")
        try:
            yield
        finally:
            n = lib.axon_stop_nrt_profile(str(output_dir).encode())
            if n < 0:
                raise RuntimeError(f"axon_stop_nrt_profile rc={n}")
            print(f"profile: {n} file(s) written to {output_dir}")

    mod = types.ModuleType("antenv.axon_hooks")
    _state = {"hook": _hook}
    mod.get_axon_ntff_profile_hook = lambda: _state["hook"]
    mod.set_axon_ntff_profile_hook = lambda h: _state.update(hook=h)
    import antenv
    sys.modules["antenv.axon_hooks"] = mod
    antenv.axon_hooks = mod


def run_spmd(in_maps, trace=False, **kw):
    nc = _get_program()
    if trace:
        _ensure_ntff_hook()
    return run_bass_kernel_spmd(nc, in_maps, list(range(N_CORES)), trace=trace, **kw)


def kernel(x, y, neg_indices):
    x = np.asarray(x)
    y = np.asarray(y)
    neg_indices = np.asarray(neg_indices)
    res = run_spmd(make_in_maps(x, y, neg_indices)).results
    losses = np.stack([res[c]["loss"] for c in range(N_CORES)])  # [8, 128, SLABS]
    return np.float32(losses.mean())



# revision 5
# speedup vs baseline: 1.5269x; 1.5269x over previous
"""Contrastive loss kernel for Trainium2, 8 NeuronCores, data-parallel over node rows.

v2 strategy (per core c, shard rows R_c = c*1024 .. c*1024+1024), gather-free:
  - Gram matrix in fp8(e4m3) with DoubleRow matmuls (K=256 folded as 128x2).
    The DR pair axis maps d = 2*q + j, which falls out of a uint16-viewed DMA
    transpose of fp8 element pairs -- no separate cast pass.
  - Mask application ON THE PE: an identity-weights fp8 matmul accumulates the
    ln-mask (0 selected / -20 unselected) straight into the Gram PSUM chunk, so
    selection is a single ACT pass: exp(2*(sim+mask)) with accum_out giving the
    per-row masked sums directly from PSUM.  A few chunks per column use the
    DVE add-first path instead to balance engine load.
  - Full-x normalize streamed in groups; rsqrt via DVE polynomial + one Newton
    step (SS/256 in [0.66,1.37]) so the Scalar activation table never thrashes
    between Sqrt and Exp mid-stream.
  - Chunk-column-major slab loop so Gram consumption tracks the group stream.
  - Positives: fused DVE dots on bf16 shard tiles; ln(pos) taken analytically.
  - loss = ln(pos + neg + eps) - 2*pos_arg; host averages the 8 cores' rows.
"""
import sys

sys.path.insert(0, "/opt/trn_rl_repo")

from contextlib import ExitStack

import numpy as np
import ml_dtypes

import concourse.bacc as bacc
import concourse.mybir as mybir
import concourse.tile as tile
from concourse.bass_utils import run_bass_kernel_spmd

N_NODES = 8192
D = 256
K_NEG = 64
N_CORES = 8
RPC = N_NODES // N_CORES      # rows per core = 1024
SLABS = RPC // 128            # 8 slabs of 128 rows
NT = N_NODES // 128           # 64 x-tiles of [128, 256]
NTS = RPC // 128              # 8 shard tiles
G = 8                         # tiles per x-group
NG = NT // G                  # 8 groups
CHUNK = 2048                  # Gram chunk (4 PSUM banks)
NCH = N_NODES // CHUNK        # 4 chunks per slab row
TAU_INV = 2.0                 # 1/(0.5 + 1e-10) ~= 2.0
EPS = 1e-5

F32 = mybir.dt.float32
BF16 = mybir.dt.bfloat16
FP8 = mybir.dt.float8e4
U16 = mybir.dt.uint16
MASK_NP = ml_dtypes.float8_e4m3

# chunks handled by the DVE add-first path (slab, chunk); all others use the
# PE-mask + ACT-only path.
V_CHUNKS = frozenset({(4, 0), (5, 1), (6, 2), (7, 3)})

_PROG = None


def _poly_rsqrt(nc, big, SSg, Rout):
    """Rout = 1/sqrt(SSg) for SSg ~ 256, DVE-only (no ACT table).

    a = SS/256; t = a-1; y0 = 1 - t/2 + 0.375 t^2; one Newton step.
    Scaled: 1/sqrt(SS) = (1/16) / sqrt(a) -> fold 1/16 into the last mul.
    """
    ALU = mybir.AluOpType
    shp = list(SSg.shape)
    t = big.tile(shp, F32, tag="rs_t")
    nc.vector.tensor_scalar(out=t, in0=SSg, scalar1=1.0 / 256.0, scalar2=-1.0,
                            op0=ALU.mult, op1=ALU.add)
    u = big.tile(shp, F32, tag="rs_u")
    nc.vector.tensor_scalar(out=u, in0=t, scalar1=0.375, scalar2=-0.5,
                            op0=ALU.mult, op1=ALU.add)
    w = big.tile(shp, F32, tag="rs_w")
    nc.vector.tensor_mul(w, u, t)
    y0 = big.tile(shp, F32, tag="rs_y0")
    nc.vector.tensor_scalar(out=y0, in0=w, scalar1=1.0, scalar2=None,
                            op0=ALU.add)
    b = big.tile(shp, F32, tag="rs_b")
    nc.vector.tensor_scalar(out=b, in0=SSg, scalar1=1.0 / 512.0, scalar2=None,
                            op0=ALU.mult)
    c = big.tile(shp, F32, tag="rs_c")
    nc.vector.tensor_mul(c, y0, y0)
    d = big.tile(shp, F32, tag="rs_d")
    nc.vector.tensor_mul(d, b, c)
    e = big.tile(shp, F32, tag="rs_e")
    nc.vector.tensor_scalar(out=e, in0=d, scalar1=-1.0, scalar2=1.5,
                            op0=ALU.mult, op1=ALU.add)
    y1 = big.tile(shp, F32, tag="rs_y1")
    nc.vector.tensor_mul(y1, y0, e)
    # R = y1 / 16
    nc.vector.tensor_scalar(out=Rout, in0=y1, scalar1=1.0 / 16.0, scalar2=None,
                            op0=ALU.mult)


def _build_program():
    nc = bacc.Bacc("TRN2", target_bir_lowering=False, debug=False,
                   num_devices=N_CORES)

    xb_d = nc.dram_tensor("xb", [N_NODES, D], BF16, kind="ExternalInput")
    xsh_d = nc.dram_tensor("xsh", [RPC, D], BF16, kind="ExternalInput")
    ys_d = nc.dram_tensor("ys", [RPC, D], BF16, kind="ExternalInput")
    lnm_d = nc.dram_tensor("lnm", [128, SLABS, N_NODES], FP8,
                           kind="ExternalInput")
    id_d = nc.dram_tensor("idf8", [128, 128], FP8, kind="ExternalInput")
    loss_d = nc.dram_tensor("loss", [128, SLABS], F32, kind="ExternalOutput")

    AF = mybir.ActivationFunctionType
    ALU = mybir.AluOpType
    DR = mybir.MatmulPerfMode.DoubleRow

    with tile.TileContext(nc) as tc, ExitStack() as ctx:
        big = ctx.enter_context(tc.tile_pool(name="big", bufs=1))
        sqpool = ctx.enter_context(tc.tile_pool(name="sqpool", bufs=1))
        zpool = ctx.enter_context(tc.tile_pool(name="zpool", bufs=2))
        ypool = ctx.enter_context(tc.tile_pool(name="ypool", bufs=2))
        expool = ctx.enter_context(tc.tile_pool(name="expool", bufs=2))
        psum = ctx.enter_context(tc.tile_pool(name="psum", bufs=2, space="PSUM"))

        # ---------------- input DMAs ----------------
        xshb = big.tile([128, NTS, D], BF16)
        nc.sync.dma_start(out=xshb,
                          in_=xsh_d.ap().rearrange("(t p) d -> p t d", p=128))
        ysb = big.tile([128, NTS, D], BF16)
        nc.sync.dma_start(out=ysb,
                          in_=ys_d.ap().rearrange("(t p) d -> p t d", p=128))
        idsb = big.tile([128, 128], FP8)
        nc.sync.dma_start(out=idsb, in_=id_d.ap())
        lnm = big.tile([128, SLABS, N_NODES], FP8)
        for s in range(SLABS):
            nc.sync.dma_start(out=lnm[:, s, :], in_=lnm_d.ap()[:, s, :])
        x_r = xb_d.ap().rearrange("(t p) d -> p t d", p=128)
        xall = big.tile([128, NT, D], BF16)
        for g in range(NG):
            nc.sync.dma_start(out=xall[:, g * G:(g + 1) * G, :],
                              in_=x_r[:, g * G:(g + 1) * G, :])

        # ---------------- shard norms + positives ----------------
        SSx = big.tile([128, NTS], F32)
        sqs = sqpool.tile([128, NTS, D], BF16, tag="sqs")
        nc.vector.tensor_mul(sqs.rearrange("p a b -> p (a b)"),
                             xshb.rearrange("p a b -> p (a b)"),
                             xshb.rearrange("p a b -> p (a b)"))
        nc.vector.tensor_reduce(out=SSx, in_=sqs, axis=mybir.AxisListType.X,
                                op=ALU.add)
        SSy = big.tile([128, NTS], F32)
        sqy = sqpool.tile([128, NTS, D], BF16, tag="sqs")
        nc.vector.tensor_mul(sqy.rearrange("p a b -> p (a b)"),
                             ysb.rearrange("p a b -> p (a b)"),
                             ysb.rearrange("p a b -> p (a b)"))
        nc.vector.tensor_reduce(out=SSy, in_=sqy, axis=mybir.AxisListType.X,
                                op=ALU.add)
        SSxi = big.tile([128, NTS], F32)
        nc.vector.reciprocal(SSxi, SSx)
        SSyi = big.tile([128, NTS], F32)
        nc.vector.reciprocal(SSyi, SSy)
        Rx = big.tile([128, NTS], F32)
        nc.scalar.activation(Rx, SSxi, AF.Sqrt)
        Ry = big.tile([128, NTS], F32)
        nc.scalar.activation(Ry, SSyi, AF.Sqrt)

        # positive dots
        xy = sqpool.tile([128, NTS, D], BF16, tag="sqs")
        nc.vector.tensor_mul(xy.rearrange("p a b -> p (a b)"),
                             xshb.rearrange("p a b -> p (a b)"),
                             ysb.rearrange("p a b -> p (a b)"))
        DXY = big.tile([128, NTS], F32)
        nc.vector.tensor_reduce(out=DXY, in_=xy, axis=mybir.AxisListType.X,
                                op=ALU.add)
        PA = big.tile([128, NTS], F32)
        nc.vector.tensor_mul(PA, DXY, Rx)
        PA2 = big.tile([128, NTS], F32)
        nc.vector.tensor_mul(PA2, PA, Ry)
        POS = big.tile([128, NTS], F32)
        nc.scalar.activation(POS, PA2, AF.Exp, scale=TAU_INV)

        # ---------------- shard z (fp8 pairs) + transpose ----------------
        zsf8 = big.tile([128, NTS, D], FP8)
        for t in range(NTS):
            nc.vector.tensor_scalar(out=zsf8[:, t, :], in0=xshb[:, t, :],
                                    scalar1=Rx[:, t:t + 1], scalar2=None,
                                    op0=ALU.mult)
        z1sT = big.tile([128, 2 * RPC], FP8)   # u16 cols = RPC
        nc.sync.dma_start(
            out=z1sT.bitcast(U16).rearrange("p (b q) -> p b q", q=128),
            in_=zsf8.bitcast(U16).rearrange("p a b -> p (a b)"),
            transpose=True)
        # de-interleave pairs for the weights: LDWEIGHTS DoubleRow requires the
        # pair stride to be a multiple of 16 elements (s3_lw_dual_fp8), so the
        # adjacent-pair transpose layout is illegal for lhsT. One DVE copy.
        z1sT_w = big.tile([128, 2, RPC], FP8)
        nc.vector.tensor_copy(out=z1sT_w,
                              in_=z1sT.rearrange("p (n j) -> p j n", j=2))

        # ---------------- full-x: norms (DVE rsqrt) + scale + transpose ----
        z1T = big.tile([128, 2 * N_NODES], FP8)  # u16 cols = N_NODES
        R = big.tile([128, NT], F32)
        SS = big.tile([128, NT], F32)
        for g in range(NG):
            sl = slice(g * G, (g + 1) * G)
            sq = sqpool.tile([128, G, D], BF16, tag="sqg")
            nc.vector.tensor_mul(sq.rearrange("p a b -> p (a b)"),
                                 xall[:, sl, :].rearrange("p a b -> p (a b)"),
                                 xall[:, sl, :].rearrange("p a b -> p (a b)"))
            nc.vector.tensor_reduce(out=SS[:, sl], in_=sq,
                                    axis=mybir.AxisListType.X, op=ALU.add)
            _poly_rsqrt(nc, zpool, SS[:, sl], R[:, sl])
            zf8 = zpool.tile([128, G, D], FP8, tag="zf8")
            for tl in range(G):
                t = g * G + tl
                nc.vector.tensor_scalar(out=zf8[:, tl, :], in0=xall[:, t, :],
                                        scalar1=R[:, t:t + 1], scalar2=None,
                                        op0=ALU.mult)
            nc.sync.dma_start(
                out=z1T.bitcast(U16)[:, g * G * 128:(g + 1) * G * 128]
                       .rearrange("p (b q) -> p b q", q=128),
                in_=zf8.bitcast(U16).rearrange("p a b -> p (a b)"),
                transpose=True)
        z1T_dr = z1T.rearrange("p (n j) -> p j n", j=2)  # [128, 2, N_NODES]

        # ---------------- selection: chunk-column-major slab loop ----------
        NEGC = big.tile([128, SLABS * NCH], F32)
        nc.vector.memset(NEGC, 0.0)
        for ch in range(NCH):
            for s in range(SLABS):
                is_v = (s, ch) in V_CHUNKS
                lhsT = z1sT_w[:, :, s * 128:(s + 1) * 128]
                ps = psum.tile([128, CHUNK], F32, tag="ps")
                for j in range(CHUNK // 512):
                    col = ch * CHUNK + j * 512
                    nc.tensor.matmul(ps[:, j * 512:(j + 1) * 512],
                                     lhsT=lhsT,
                                     rhs=z1T_dr[:, :, col:col + 512],
                                     start=True, stop=is_v, perf_mode=DR)
                if is_v:
                    Y = ypool.tile([128, CHUNK], BF16, tag="y")
                    nc.vector.tensor_tensor(
                        out=Y, in0=ps,
                        in1=lnm[:, s, ch * CHUNK:(ch + 1) * CHUNK],
                        op=ALU.add)
                    ex = expool.tile([128, CHUNK], BF16, tag="ex")
                    nc.scalar.activation(
                        ex, Y, AF.Exp, scale=TAU_INV,
                        accum_out=NEGC[:, s * NCH + ch:s * NCH + ch + 1])
                else:
                    for j in range(CHUNK // 512):
                        col = ch * CHUNK + j * 512
                        nc.tensor.matmul(ps[:, j * 512:(j + 1) * 512],
                                         lhsT=idsb,
                                         rhs=lnm[:, s, col:col + 512],
                                         start=False, stop=True)
                    ex = expool.tile([128, CHUNK], BF16, tag="ex")
                    nc.scalar.activation(
                        ex, ps, AF.Exp, scale=TAU_INV,
                        accum_out=NEGC[:, s * NCH + ch:s * NCH + ch + 1])

        # ---------------- loss assembly ----------------
        NEG = big.tile([128, SLABS], F32)
        nc.vector.tensor_reduce(
            out=NEG, in_=NEGC.rearrange("p (s c) -> p s c", c=NCH),
            axis=mybir.AxisListType.X, op=ALU.add)
        DEN = big.tile([128, SLABS], F32)
        nc.vector.tensor_add(DEN, NEG, POS)
        DEN2 = big.tile([128, SLABS], F32)
        nc.vector.tensor_scalar_add(DEN2, DEN, EPS)
        LD = big.tile([128, SLABS], F32)
        nc.scalar.activation(LD, DEN2, AF.Ln)
        LP = big.tile([128, SLABS], F32)
        nc.vector.tensor_scalar_mul(LP, PA2, TAU_INV)
        LOSS = big.tile([128, SLABS], F32)
        nc.vector.tensor_sub(LOSS, LD, LP)
        nc.sync.dma_start(out=loss_d.ap(), in_=LOSS)

    nc.compile()
    return nc


def _get_program():
    global _PROG
    if _PROG is None:
        _PROG = _build_program()
    return _PROG


def _make_mask(idx_core: np.ndarray) -> np.ndarray:
    """[1024, 64] int -> [128, 8, 8192] fp8 ln-mask (p, slab, col)."""
    idxc = idx_core.reshape(SLABS, 128, K_NEG).transpose(1, 0, 2)  # [p, s, k]
    lnm = np.full((128, SLABS, N_NODES), -20.0, dtype=np.float32)
    pp = np.arange(128)[:, None, None]
    ss = np.arange(SLABS)[None, :, None]
    lnm[pp, ss, idxc] = 0.0
    return lnm.astype(MASK_NP)


def make_in_maps(x, y, neg_indices):
    xb = np.ascontiguousarray(x).astype(ml_dtypes.bfloat16)
    idf8 = np.eye(128, dtype=np.float32).astype(MASK_NP)
    in_maps = []
    for c in range(N_CORES):
        lo, hi = c * RPC, (c + 1) * RPC
        in_maps.append({
            "xb": xb,
            "xsh": xb[lo:hi],
            "ys": np.ascontiguousarray(y[lo:hi]).astype(ml_dtypes.bfloat16),
            "lnm": _make_mask(neg_indices[lo:hi]),
            "idf8": idf8,
        })
    return in_maps


def _ensure_ntff_hook():
    """Register the axon NTFF profile hook (missing from this image's antenv)."""
    import types, ctypes, contextlib
    try:
        from antenv.axon_hooks import get_axon_ntff_profile_hook  # noqa
        return
    except ImportError:
        pass
    so_path = "/opt/axon/libaxon_pjrt.so"
    import os
    if not os.path.exists(so_path):
        return
    lib = ctypes.CDLL(so_path)
    if not hasattr(lib, "axon_start_nrt_profile"):
        return
    lib.axon_start_nrt_profile.argtypes = [ctypes.POINTER(ctypes.c_int64),
                                           ctypes.c_size_t]
    lib.axon_start_nrt_profile.restype = ctypes.c_int64
    lib.axon_stop_nrt_profile.argtypes = [ctypes.c_char_p]
    lib.axon_stop_nrt_profile.restype = ctypes.c_int64

    @contextlib.contextmanager
    def _hook(output_dir, device_ids):
        import jax
        jax.devices()
        if device_ids:
            ids = (ctypes.c_int64 * len(device_ids))(*device_ids)
            rc = lib.axon_start_nrt_profile(ids, len(device_ids))
        else:
            rc = lib.axon_start_nrt_profile(None, 0)
        if rc != 0:
            raise RuntimeError(f"axon_start_nrt_profile rc={rc}")
        try:
            yield
        finally:
            n = lib.axon_stop_nrt_profile(str(output_dir).encode())
            if n < 0:
                raise RuntimeError(f"axon_stop_nrt_profile rc={n}")
            print(f"profile: {n} file(s) written to {output_dir}")

    mod = types.ModuleType("antenv.axon_hooks")
    _state = {"hook": _hook}
    mod.get_axon_ntff_profile_hook = lambda: _state["hook"]
    mod.set_axon_ntff_profile_hook = lambda h: _state.update(hook=h)
    import antenv
    sys.modules["antenv.axon_hooks"] = mod
    antenv.axon_hooks = mod


def run_spmd(in_maps, trace=False, **kw):
    nc = _get_program()
    if trace:
        _ensure_ntff_hook()
    return run_bass_kernel_spmd(nc, in_maps, list(range(N_CORES)), trace=trace, **kw)


def kernel(x, y, neg_indices):
    x = np.asarray(x)
    y = np.asarray(y)
    neg_indices = np.asarray(neg_indices)
    res = run_spmd(make_in_maps(x, y, neg_indices)).results
    losses = np.stack([res[c]["loss"] for c in range(N_CORES)])  # [8, 128, SLABS]
    return np.float32(losses.mean())
